# revision 1
# baseline (speedup 1.0000x reference)
"""Trainium2 Bass kernel for nn_CrossAttention (MLA-style cross attention).

Sharding: 8 cores = 2 batches x 4 head-groups (4 heads each).
Per core: replicated down-projections (c_q, c_kv), per-head up/rope
projections, attention, and a row-shard of the final fc; the host sums the 4
partial fc outputs per batch.

All device activations are feature-major [dims, seq] so matmuls contract over
the partition dim. Scores are computed transposed (k on partitions, q on free)
which makes softmax-normalization sums come out of the PV matmul via an
appended ones-column in V; no max-subtraction is needed (scores/16 is O(1) for
this problem's scale). RoPE is applied via a second "partner" matmul whose
weights are the sign-flipped pair-swapped columns, combined with host-built
sin/cos tables on the vector engine.
"""

import math
from contextlib import ExitStack

import numpy as np
import ml_dtypes

import concourse.bass as bass
import concourse.tile as tile
from concourse import bacc, mybir
from concourse.bass_utils import run_bass_kernel_spmd

bf16 = ml_dtypes.bfloat16
F32 = mybir.dt.float32
BF = mybir.dt.bfloat16

# problem constants (hardcoded per contract)
B, S, Z, DOWN, UP, H, RHD, VHD = 2, 2048, 1024, 512, 1024, 16, 64, 64
HPC = 4            # heads per core
NCORES = 8
SCALE = 1.0 / (math.sqrt(64) + math.sqrt(64))  # 1/16

_cache = {}


def _rope_tables():
    theta = 1.0 / (10000.0 ** (np.arange(0, RHD, 2, dtype=np.float32) / RHD))
    pe = np.arange(S, dtype=np.float32)[:, None] * theta[None, :]
    # faithful to reference: cos_pos stores sin, sin_pos stores cos
    cos_pos = np.repeat(np.sin(pe), 2, axis=-1).T.astype(np.float32)  # [RHD, S]
    sin_pos = np.repeat(np.cos(pe), 2, axis=-1).T.astype(np.float32)
    return cos_pos, sin_pos


def _partner_cols(w):
    """wp[:, 2i] = -w[:, 2i+1]; wp[:, 2i+1] = w[:, 2i]"""
    wp = np.empty_like(w)
    wp[..., 0::2] = -w[..., 1::2]
    wp[..., 1::2] = w[..., 0::2]
    return wp


def build_nc(with_bias):
    nc = bacc.Bacc("TRN2", target_bir_lowering=False, debug=False,
                   num_devices=NCORES)

    def din(name, shape, dt=BF):
        return nc.dram_tensor(name, shape, dt, kind="ExternalInput").ap()

    qT = din("qT", [Z, S])
    kT = din("kT", [Z, S])
    wdq = din("wdq", [Z, DOWN])
    wdkv = din("wdkv", [Z, DOWN])
    w1 = din("w1", [DOWN, 512])     # per head [w_uq_h | w_qr_h]
    w2 = din("w2", [DOWN, 256])     # packed partner(w_qr) head cols
    wk = din("wk", [DOWN, 256])     # packed w_uk head cols
    wv = din("wv", [DOWN, 256])
    wkr2 = din("wkr2", [Z, 128])    # [partner(w_kr) | w_kr]
    ct1 = din("ct1", [128, S])  # rows 0:64 ones, 64:128 cos_pos.T
    st1 = din("st1", [128, S])  # sin_pos.T stacked in both row halves
    wfc = din("wfc", [256, Z])
    if with_bias:
        bdq = din("bdq", [128, 4], F32)    # b_dq chunked per M-chunk
        bdkv = din("bdkv", [128, 4], F32)
        biasq = din("biasq", [512, S])  # per-head qcat bias contribution
        biask = din("biask", [512, S])  # per-head kcat bias contribution
    outT = nc.dram_tensor("outT", [Z, S], F32, kind="ExternalOutput").ap()

    with tile.TileContext(nc) as tc, ExitStack() as ctx:
        # ---- persistent pools ----
        sp = ctx.enter_context(tc.tile_pool(name="static", bufs=1))

        def stile(shape, dt, name):
            return sp.tile(shape, dt, name=name, tag=name)

        wdq_sb = stile([128, 8, DOWN], BF, "wdq_sb")
        wdkv_sb = stile([128, 8, DOWN], BF, "wdkv_sb")
        w1_sb = stile([128, 4, 512], BF, "w1_sb")
        w2_sb = stile([128, 4, 256], BF, "w2_sb")
        wk_sb = stile([128, 4, 256], BF, "wk_sb")
        wv_sb = stile([128, 4, 256], BF, "wv_sb")
        wkr2_sb = stile([128, 8, 128], BF, "wkr2_sb")
        wfc_sb = stile([128, 2, 8, 128], BF, "wfc_sb")
        ct_sb = stile([128, S], BF, "ct_sb")
        st_sb = stile([128, S], BF, "st_sb")
        nc.sync.dma_start(wdq_sb[:], wdq.rearrange("(c p) m -> p c m", p=128))
        nc.sync.dma_start(wdkv_sb[:], wdkv.rearrange("(c p) m -> p c m", p=128))
        nc.sync.dma_start(wkr2_sb[:], wkr2.rearrange("(c p) m -> p c m", p=128))
        if with_bias:
            bdq_sb = stile([128, 4], F32, "bdq_sb")
            bdkv_sb = stile([128, 4], F32, "bdkv_sb")
            nc.sync.dma_start(bdq_sb[:], bdq[:])
            nc.sync.dma_start(bdkv_sb[:], bdkv[:])

        cq_sb = stile([128, 4, S], BF, "cq_sb")      # c_q^T
        ckv_sb = stile([128, 4, S], BF, "ckv_sb")    # c_kv^T
        qcat_sb = stile([128, 4, S], BF, "qcat_sb")  # per head [128, S]
        kcat_sb = stile([128, 4, S], BF, "kcat_sb")
        va_sb = stile([128, 16, HPC * 65], BF, "va_sb")  # v_aug per s-chunk
        af_sb = stile([128, 2, S], BF, "af_sb")      # fc rhs (attn out)
        tmpa_sb = stile([128, S], BF, "tmpa_sb")     # k-rope raw
        tmpb_sb = stile([128, S], BF, "tmpb_sb")     # k-rope partner shifted

        # ======== phase 1: down projections ========
        pps = ctx.enter_context(tc.tile_pool(name="pps", bufs=2, space="PSUM"))

        def psA():
            return pps.tile([128, 1024], F32, name="psA", tag="psA")

        def psB():
            return pps.tile([128, 1024], F32, name="psB", tag="psB")

        with tc.tile_pool(name="qk_stream", bufs=1) as qkp:
            qT_sb = qkp.tile([128, 8, S], BF, name="qT_sb", tag="qT_sb")
            kT_sb = qkp.tile([128, 8, S], BF, name="kT_sb", tag="kT_sb")
            qT_r = qT.rearrange("(c p) s -> p c s", p=128)
            kT_r = kT.rearrange("(c p) s -> p c s", p=128)
            # chunked loads so the first matmuls start after ~1 chunk;
            # first S-halves of all z-chunks land first (first accum groups
            # only need sh0)
            for sh in range(2):
                ssl = slice(1024 * sh, 1024 * (sh + 1))
                for zc in range(8):
                    nc.sync.dma_start(qT_sb[:, zc, ssl], qT_r[:, zc, ssl])
                for zc in range(8):
                    nc.sync.dma_start(kT_sb[:, zc, ssl], kT_r[:, zc, ssl])

            # phase-2+ weights arrive while phase 1 computes
            nc.sync.dma_start(w1_sb[:], w1.rearrange("(c p) m -> p c m", p=128))
            nc.sync.dma_start(w2_sb[:], w2.rearrange("(c p) m -> p c m", p=128))
            nc.sync.dma_start(wk_sb[:], wk.rearrange("(c p) m -> p c m", p=128))
            nc.sync.dma_start(wv_sb[:], wv.rearrange("(c p) m -> p c m", p=128))
            nc.sync.dma_start(ct_sb[:], ct1[:])
            nc.sync.dma_start(st_sb[:], st1[:])
            nc.sync.dma_start(wfc_sb[:],
                              wfc.rearrange("(c p) (z m) -> p c z m",
                                            p=128, m=128))

            for (src, wsb, dst, bcol) in (
                (qT_sb, wdq_sb, cq_sb, "q"),
                (kT_sb, wdkv_sb, ckv_sb, "kv"),
            ):
                for sf in range(4):
                    for m in range(4):
                        ps = (psA if (m + sf) % 2 == 0 else psB)()[:, 0:512]
                        for zc in range(8):
                            nc.tensor.matmul(
                                ps[:], wsb[:, zc, 128 * m:128 * (m + 1)],
                                src[:, zc, 512 * sf:512 * (sf + 1)],
                                start=(zc == 0), stop=(zc == 7))
                        dd = dst[:, m, 512 * sf:512 * (sf + 1)]
                        if not with_bias:
                            nc.scalar.copy(dd, ps[:])
                        elif with_bias:
                            bs = bdq_sb if bcol == "q" else bdkv_sb
                            nc.vector.tensor_tensor(
                                dd, ps[:], bs[:, m:m + 1].to_broadcast([128, 512]),
                                mybir.AluOpType.add)

            # k-rope raw: psum = [partner(w_kr) | w_kr]^T @ kT
            if True:
                for sf in range(4):
                    ps = psB()[:, 0:512]
                    for zc in range(8):
                        nc.tensor.matmul(
                            ps[:], wkr2_sb[:, zc, :],
                            kT_sb[:, zc, 512 * sf:512 * (sf + 1)],
                            start=(zc == 0), stop=(zc == 7))
                    nc.scalar.copy(tmpa_sb[:, 512 * sf:512 * (sf + 1)], ps[:])
            # shift partner rows 0:64 -> tmpb rows 64:128
            nc.sync.dma_start(tmpb_sb[64:128, :], tmpa_sb[0:64, :])

        if with_bias:
            bias_pool = ctx.enter_context(tc.tile_pool(name="bias_pool", bufs=1))
            biasq_sb = bias_pool.tile([128, 4, S], BF, name="biasq_sb", tag="biasq_sb")
            biask_sb = bias_pool.tile([128, 4, S], BF, name="biask_sb", tag="biask_sb")
            nc.sync.dma_start(biasq_sb[:], biasq.rearrange("(c p) s -> p c s", p=128))
            nc.sync.dma_start(biask_sb[:], biask.rearrange("(c p) s -> p c s", p=128))

        # ======== attention (emitted per head-pair, interleaved) ========
        wrk3 = ctx.enter_context(tc.tile_pool(name="wrk3", bufs=3))
        va_v = va_sb.rearrange("p sc (h e) -> p sc h e", e=65)

        def attention(h):
            for qh in range(2):
                pv = psB()
                for kc in range(16):
                    sc_ps = psA()
                    for half in range(2):
                        psl = slice(512 * half, 512 * (half + 1))
                        rsl = slice(1024 * qh + 512 * half,
                                    1024 * qh + 512 * (half + 1))
                        nc.tensor.matmul(
                            sc_ps[:, psl],
                            kcat_sb[:, h, 128 * kc:128 * (kc + 1)],
                            qcat_sb[:, h, rsl], start=True, stop=True)
                    pr = wrk3.tile([128, 1024], BF, name="pr", tag="pr",
                                   bufs=3 if with_bias else 8)
                    nc.scalar.activation(pr[:], sc_ps[:],
                                         mybir.ActivationFunctionType.Exp,
                                         scale=SCALE)
                    for half in range(2):
                        psl = slice(512 * half, 512 * (half + 1))
                        nc.tensor.matmul(
                            pv[0:65, psl], va_v[:, kc, h, :], pr[:, psl],
                            start=(kc == 0), stop=(kc == 15))
                qsl = slice(1024 * qh, 1024 * (qh + 1))
                srow = wrk3.tile([1, 1024], F32, name="srow", tag="srow", bufs=2 if with_bias else 3)
                nc.vector.tensor_copy(srow[:], pv[64:65, :])
                rec = wrk3.tile([1, 1024], F32, name="rec", tag="rec")
                nc.vector.reciprocal_approx_fast(rec[:], srow[:])
                bc = wrk3.tile([64, 1024], F32, name="bc", tag="bc", bufs=2 if with_bias else 3)
                nc.gpsimd.partition_broadcast(bc[:], rec[:])
                ro = slice(0, 64) if h % 2 == 0 else slice(64, 128)
                nc.vector.tensor_tensor(af_sb[ro, h // 2, qsl],
                                        pv[0:64, :], bc[:],
                                        mybir.AluOpType.mult)

        # ======== phase 2: up projections, qcat/kcat/v assembly ========
        with tc.tile_pool(name="wrk2", bufs=3) as wrk2:
            # kcat rows 64:128 = rope(k): combine tmpa/tmpb into kcat head 0
            k0 = kcat_sb[64:128, 0, :]
            tt2 = wrk2.tile([128, S], BF, name="tt2", tag="tt2", bufs=1 if with_bias else 3)
            nc.vector.tensor_tensor(k0, tmpa_sb[64:128, :], ct_sb[64:128, :],
                                    mybir.AluOpType.mult)
            nc.vector.tensor_tensor(tt2[64:128, :], tmpb_sb[64:128, :],
                                    st_sb[64:128, :], mybir.AluOpType.mult)
            nc.vector.tensor_tensor(k0, k0, tt2[64:128, :], mybir.AluOpType.add)
            if with_bias:
                nc.vector.tensor_tensor(k0, k0, biask_sb[64:128, 0, :],
                                        mybir.AluOpType.add)
            for h in range(1, HPC):
                kd = kcat_sb[64:128, h, :]
                nc.vector.tensor_copy(kd, k0)
                if with_bias:
                    # head-0 table already added; biask rope rows are shared,
                    # so the copy carries them. (rows 64:128 identical per head)
                    pass

            # qcat: A (w1) per head + packed B (w2, head pairs) rope-combined
            for sf in range(2):
                ssl = slice(1024 * sf, 1024 * (sf + 1))
                for pair in range(0, 1):
                    pb = psB()
                    for half in range(2):
                        hsl = slice(1024 * sf + 512 * half,
                                    1024 * sf + 512 * (half + 1))
                        psl = slice(512 * half, 512 * (half + 1))
                        for dc in range(4):
                            nc.tensor.matmul(
                                pb[:, psl],
                                w2_sb[:, dc, 128 * pair:128 * (pair + 1)],
                                cq_sb[:, dc, hsl], start=(dc == 0), stop=(dc == 3))
                    for sub in range(2):
                        h = 2 * pair + sub
                        rsl = slice(64 * sub, 64 * (sub + 1))
                        pa = psA()
                        for half in range(2):
                            hsl = slice(1024 * sf + 512 * half,
                                        1024 * sf + 512 * (half + 1))
                            psl = slice(512 * half, 512 * (half + 1))
                            for dc in range(4):
                                nc.tensor.matmul(
                                    pa[:, psl],
                                    w1_sb[:, dc, 128 * h:128 * (h + 1)],
                                    cq_sb[:, dc, hsl],
                                    start=(dc == 0), stop=(dc == 3))
                        qd = qcat_sb[:, h, ssl]
                        tt = wrk2.tile([128, 1024], BF, name="tt", tag="tt", bufs=2 if with_bias else 3)
                        nc.vector.tensor_tensor(qd, pa[:], ct_sb[:, ssl],
                                                mybir.AluOpType.mult)
                        # rope partner: pb rows 64*sub.. x st rows (same base)
                        nc.vector.tensor_tensor(
                            tt[64:128, :], pb[rsl, :], st_sb[rsl, ssl],
                            mybir.AluOpType.mult)
                        nc.vector.tensor_tensor(qd[64:128, :], qd[64:128, :],
                                                tt[64:128, :],
                                                mybir.AluOpType.add)
                        if with_bias:
                            nc.vector.tensor_tensor(qd, qd, biasq_sb[:, h, ssl],
                                                    mybir.AluOpType.add)

            # kcat rows 0:64 = k_t_c, head pairs packed; odd heads copy-shift
            for pair in range(0, 1):
                for sf in range(2):
                    ssl = slice(1024 * sf, 1024 * (sf + 1))
                    pk = psA()
                    for half in range(2):
                        hsl = slice(1024 * sf + 512 * half,
                                    1024 * sf + 512 * (half + 1))
                        psl = slice(512 * half, 512 * (half + 1))
                        for dc in range(4):
                            nc.tensor.matmul(
                                pk[:, psl],
                                wk_sb[:, dc, 128 * pair:128 * (pair + 1)],
                                ckv_sb[:, dc, hsl], start=(dc == 0), stop=(dc == 3))
                    for sub in range(2):
                        h = 2 * pair + sub
                        kd = kcat_sb[0:64, h, ssl]
                        psrc = pk[64 * sub:64 * (sub + 1), :]
                        if with_bias:
                            nc.vector.tensor_copy(kd, psrc)
                        else:
                            nc.scalar.copy(kd, psrc)
                        if with_bias:
                            nc.vector.tensor_tensor(kd, kd,
                                                    biask_sb[0:64, h, ssl],
                                                    mybir.AluOpType.add)

            # v_aug: ones cols then strided copies of v_nat
            nc.vector.memset(va_sb[:, :, 64::65], 1.0)
            for sc in range(16):
                pv_ = psB()[:, 0:256]
                for dc in range(4):
                    nc.tensor.matmul(
                        pv_[:], ckv_sb[:, dc, 128 * sc:128 * (sc + 1)],
                        wv_sb[:, dc, :], start=(dc == 0), stop=(dc == 3))
                dst = va_sb[:, sc, :].rearrange("p (h e) -> p h e", e=65)[:, :, 0:64]
                nc.scalar.copy(dst, pv_[:].rearrange("p (h e) -> p h e", e=64))

            # qcat: A (w1) per head + packed B (w2, head pairs) rope-combined
            for sf in range(2):
                ssl = slice(1024 * sf, 1024 * (sf + 1))
                for pair in range(1, 2):
                    pb = psB()
                    for half in range(2):
                        hsl = slice(1024 * sf + 512 * half,
                                    1024 * sf + 512 * (half + 1))
                        psl = slice(512 * half, 512 * (half + 1))
                        for dc in range(4):
                            nc.tensor.matmul(
                                pb[:, psl],
                                w2_sb[:, dc, 128 * pair:128 * (pair + 1)],
                                cq_sb[:, dc, hsl], start=(dc == 0), stop=(dc == 3))
                    for sub in range(2):
                        h = 2 * pair + sub
                        rsl = slice(64 * sub, 64 * (sub + 1))
                        pa = psA()
                        for half in range(2):
                            hsl = slice(1024 * sf + 512 * half,
                                        1024 * sf + 512 * (half + 1))
                            psl = slice(512 * half, 512 * (half + 1))
                            for dc in range(4):
                                nc.tensor.matmul(
                                    pa[:, psl],
                                    w1_sb[:, dc, 128 * h:128 * (h + 1)],
                                    cq_sb[:, dc, hsl],
                                    start=(dc == 0), stop=(dc == 3))
                        qd = qcat_sb[:, h, ssl]
                        tt = wrk2.tile([128, 1024], BF, name="tt", tag="tt", bufs=2 if with_bias else 3)
                        nc.vector.tensor_tensor(qd, pa[:], ct_sb[:, ssl],
                                                mybir.AluOpType.mult)
                        # rope partner: pb rows 64*sub.. x st rows (same base)
                        nc.vector.tensor_tensor(
                            tt[64:128, :], pb[rsl, :], st_sb[rsl, ssl],
                            mybir.AluOpType.mult)
                        nc.vector.tensor_tensor(qd[64:128, :], qd[64:128, :],
                                                tt[64:128, :],
                                                mybir.AluOpType.add)
                        if with_bias:
                            nc.vector.tensor_tensor(qd, qd, biasq_sb[:, h, ssl],
                                                    mybir.AluOpType.add)

            # kcat rows 0:64 = k_t_c, head pairs packed; odd heads copy-shift
            for pair in range(1, 2):
                for sf in range(2):
                    ssl = slice(1024 * sf, 1024 * (sf + 1))
                    pk = psA()
                    for half in range(2):
                        hsl = slice(1024 * sf + 512 * half,
                                    1024 * sf + 512 * (half + 1))
                        psl = slice(512 * half, 512 * (half + 1))
                        for dc in range(4):
                            nc.tensor.matmul(
                                pk[:, psl],
                                wk_sb[:, dc, 128 * pair:128 * (pair + 1)],
                                ckv_sb[:, dc, hsl], start=(dc == 0), stop=(dc == 3))
                    for sub in range(2):
                        h = 2 * pair + sub
                        kd = kcat_sb[0:64, h, ssl]
                        psrc = pk[64 * sub:64 * (sub + 1), :]
                        if with_bias:
                            nc.vector.tensor_copy(kd, psrc)
                        else:
                            nc.scalar.copy(kd, psrc)
                        if with_bias:
                            nc.vector.tensor_tensor(kd, kd,
                                                    biask_sb[0:64, h, ssl],
                                                    mybir.AluOpType.add)

            attention(0)
            attention(1)
            attention(2)
            attention(3)

        # ======== phase 4: fc ========
        with tc.tile_pool(name="wrk4", bufs=4) as wrk4:
            for qf in range(4):
                for zc in range(8):
                    qsl = slice(512 * qf, 512 * (qf + 1))
                    fp = (psA if zc % 2 == 0 else psB)()[:, 0:512]
                    for c in range(2):
                        nc.tensor.matmul(fp[:], wfc_sb[:, c, zc, :],
                                         af_sb[:, c, qsl],
                                         start=(c == 0), stop=(c == 1))
                    ob = wrk4.tile([128, 512], F32, name="ob", tag="ob", bufs=3 if with_bias else 4)
                    if zc % 2 == 0:
                        nc.vector.tensor_copy(ob[:], fp[:])
                    else:
                        nc.scalar.copy(ob[:], fp[:])
                    nc.sync.dma_start(outT[128 * zc:128 * (zc + 1), qsl], ob[:])

    nc.compile()
    return nc


def _prep_in_maps(inputs):
    f32 = np.float32
    q = np.asarray(inputs["query"], f32)
    k = np.asarray(inputs["key"], f32)
    w_dq = np.asarray(inputs["w_dq"], f32)
    w_dkv = np.asarray(inputs["w_dkv"], f32)
    w_uq = np.asarray(inputs["w_uq"], f32)
    w_uk = np.asarray(inputs["w_uk"], f32)
    w_uv = np.asarray(inputs["w_uv"], f32)
    w_qr = np.asarray(inputs["w_qr"], f32)
    w_kr = np.asarray(inputs["w_kr"], f32)
    w_fc = np.asarray(inputs["w_fc"], f32)
    b_dq = np.asarray(inputs["b_dq"], f32)
    b_dkv = np.asarray(inputs["b_dkv"], f32)
    b_uq = np.asarray(inputs["b_uq"], f32)
    b_uk = np.asarray(inputs["b_uk"], f32)
    b_qr = np.asarray(inputs["b_qr"], f32)
    b_kr = np.asarray(inputs["b_kr"], f32)

    CT, ST = _rope_tables()
    ct1 = np.concatenate([np.ones((64, S), f32), CT], axis=0)
    st1 = np.concatenate([ST, ST], axis=0)

    with_bias = any(np.any(np.asarray(inputs[n])) for n in
                    ("b_dq", "b_dkv", "b_uq", "b_uk", "b_qr", "b_kr"))

    qTb = [q[b_].T.astype(bf16) for b_ in range(B)]
    kTb = [k[b_].T.astype(bf16) for b_ in range(B)]

    in_maps = []
    for core in range(NCORES):
        b_idx, grp = core // HPC, core % HPC
        h0 = HPC * grp
        hsl = slice(64 * h0, 64 * (h0 + HPC))
        W1 = np.zeros((DOWN, 512), f32)
        W2 = np.zeros((DOWN, 256), f32)
        Wk = np.zeros((DOWN, 256), f32)
        for i in range(HPC):
            hh = h0 + i
            W1[:, 128 * i:128 * i + 64] = w_uq[:, 64 * hh:64 * hh + 64]
            W1[:, 128 * i + 64:128 * (i + 1)] = w_qr[:, 64 * hh:64 * hh + 64]
            W2[:, 64 * i:64 * (i + 1)] = _partner_cols(
                w_qr[:, 64 * hh:64 * hh + 64])
            Wk[:, 64 * i:64 * (i + 1)] = w_uk[:, 64 * hh:64 * hh + 64]
        m = {
            "qT": qTb[b_idx], "kT": kTb[b_idx],
            "wdq": w_dq.astype(bf16), "wdkv": w_dkv.astype(bf16),
            "w1": W1.astype(bf16), "w2": W2.astype(bf16),
            "wk": Wk.astype(bf16), "wv": w_uv[:, hsl].astype(bf16),
            "wkr2": np.concatenate([_partner_cols(w_kr), w_kr],
                                   axis=1).astype(bf16),
            "ct1": ct1.astype(bf16), "st1": st1.astype(bf16),
            "wfc": w_fc[hsl, :].astype(bf16),
        }
        if with_bias:
            bq = np.zeros((512, S), f32)
            bk = np.zeros((512, S), f32)
            for i in range(HPC):
                hh = h0 + i
                bq[128 * i:128 * i + 64] = b_uq[64 * hh:64 * hh + 64, None]
                bq[128 * i + 64:128 * (i + 1)] = (
                    b_qr[64 * hh:64 * hh + 64, None] * CT
                    + _partner_cols(b_qr[None, 64 * hh:64 * hh + 64])[0][:, None] * ST)
                bk[128 * i:128 * i + 64] = b_uk[64 * hh:64 * hh + 64, None]
                bk[128 * i + 64:128 * (i + 1)] = (
                    b_kr[:, None] * CT
                    + _partner_cols(b_kr[None, :])[0][:, None] * ST)
            m["bdq"] = b_dq.reshape(4, 128).T.copy()
            m["bdkv"] = b_dkv.reshape(4, 128).T.copy()
            m["biasq"] = bq.astype(bf16)
            m["biask"] = bk.astype(bf16)
        in_maps.append(m)
    return in_maps, with_bias


def kernel(**inputs) -> np.ndarray:
    in_maps, with_bias = _prep_in_maps(inputs)

    key = ("nc", with_bias)
    if key not in _cache:
        _cache[key] = build_nc(with_bias)
    nc = _cache[key]

    res = run_bass_kernel_spmd(nc, in_maps, core_ids=list(range(NCORES)))

    f32 = np.float32
    out = np.zeros((B, S, Z), f32)
    for core in range(NCORES):
        out[core // HPC] += res.results[core]["outT"].T

    bias = (np.asarray(inputs["b_fc"], f32)
            + np.asarray(inputs["b_uv"], f32) @ np.asarray(inputs["w_fc"], f32))
    out += bias[None, None, :]
    return out.astype(np.float32)

